# revision 20
# baseline (speedup 1.0000x reference)
"""AttentionalJoin kernel for 8 Trainium2 NeuronCores.

Math: the reference builds full (M x M) self-attention over M = N+1 tokens
(CLS prepended) but returns only the CLS row of the projected output.  Only
the CLS query survives, so attention collapses to a softmax-weighted token
pooling:

    q       = Wq @ cls                       (per head h: q_h)
    score_t = scale * q_h . (Wk x_t)_h  =  x_t . R[:, h],   R = scale*Wk_h^T q_h
    p       = softmax over the M tokens
    pooled_h = sum_t p_t x_t                 (linearity: project AFTER pooling)
    out     = proj( concat_h Wv_h pooled_h ) + proj_b

Device design (per core, 2 batches of 2048 tokens):
  * x is uploaded TWICE in fp8-e3m4 (scaled by 2 to dodge the e3m4 subnormal
    cliff): once c-major (xt, the scores operand) and once t-major with an
    appended ones column (xn, the pooling operand + partition function Z).
    Dual-layout upload removes every on-chip transpose: the baseline spent
    ~60% of its time building X^T on the PE and copying PSUM->SBUF.
  * scores: per 128-token block, xt blocks are the PE's STATIONARY operand
    (fast-weight-load, 128-col fp8) against a tiny bf16 rhs R[128c, 8h],
    accumulating E-layout [128t, 8h] directly in PSUM.  Mixed-dtype matmul
    (fp8 weights x bf16 moving) is supported by the PE and keeps R at bf16
    precision for free.
  * exp: one ACT activation per batch over [128, 16*8] PSUM -> bf16 E.
  * pooling: E blocks [128t, 8h] are the stationary operand (bf16) against
    the streamed xn [128t, 260] halves (fp8), accumulating pooled^T [8, 260]
    over the 16 blocks of each batch.  Column 256 is the ones column -> Z.
  * tail (CLS contribution, normalize, Wv head-mix, proj, bias: ~10 MFLOP)
    runs on host in fp64.

Sharding: data-parallel over the batch dim, 2 batches per core.
"""

import numpy as np

H = 8
C = 512
HD = C // H
B = 16
N = 2048
NCORES = 8
BPC = B // NCORES          # batches per core
TOK = BPC * N              # tokens per core (4096)
NBLK = TOK // 128          # 128-token blocks per core (32)
NBB = N // 128             # blocks per batch (16)
HALF = 260                 # pooling rhs half width: 256 x + 1 ones + 3 pad
XSC = 2.0                  # x is stored as 2*x in e3m4
MAX_DRAIN_WAITS = 1        # this walrus rejects instructions w/ >1 sem wait

_cached = {}


def _patch_drain():
    """The container's walrus codegen rejects instructions carrying more
    than one sem wait ("Too many sync wait commands").  Split extra waits
    onto dedicated same-engine NOPs, which preserves semantics (engine
    queues are in-order)."""
    import concourse.tile as tile_mod
    from concourse import mybir
    from bass_rust import ScopedClock

    if getattr(tile_mod.TileContext, "_drain_patched", False):
        return

    orig_lower = tile_mod.TileContext._lower_ordered_insts

    def _lower_ordered_insts(self, ordered):
        nc = self.nc
        for bbname, insts in ordered.items():
            out = []
            for inst in insts:
                si = inst.sync_info
                if si is not None and si.on_wait and len(si.on_wait) > MAX_DRAIN_WAITS:
                    waits = list(si.on_wait)
                    extra, keep = waits[:-MAX_DRAIN_WAITS], waits[-MAX_DRAIN_WAITS:]
                    for w in extra:
                        nop = mybir.InstNoOp(
                            name=f"waitsplit-{nc.next_id()}",
                            engine=inst.engine,
                            ins=[],
                            outs=[],
                            bass_nofuse=True,
                            sync_info=mybir.SyncInfo(on_wait=[w], on_update=[]),
                            debug=inst.debug,
                        )
                        out.append(nop)
                    inst.sync_info = mybir.SyncInfo(
                        on_wait=keep, on_update=list(si.on_update)
                    )
                out.append(inst)
            ordered[bbname] = out
        return orig_lower(self, ordered)

    tile_mod.TileContext._lower_ordered_insts = _lower_ordered_insts

    def _drain_and_barrier(self, tick_clock, wait_clock):
        nc = self.nc
        probe = mybir.InstNoOp(
            name=f"drain-wait-probe-{nc.next_id()}",
            engine=mybir.EngineType.SP,
            ins=[],
            outs=[],
        )
        wait_clock.add_sem_waits(probe, ScopedClock({None: tick_clock.global_clock}))
        waits = list(probe.sync_info.on_wait) if probe.sync_info else []
        for i in range(0, len(waits), MAX_DRAIN_WAITS):
            chunk = waits[i : i + MAX_DRAIN_WAITS]
            nop = nc.sync.nop(nofuse=True, hint="drain_wait")
            nop.ins.sync_info = mybir.SyncInfo(on_wait=chunk, on_update=[])
        nc.sync.drain()

        nc.all_engine_barrier()
        popped = nc._tile_sem_poison_stack.pop()
        assert popped is self._sem_poison
        nc.clear_and_free_semaphores(list(self.sems.allocated().values()))
        nc.all_engine_barrier()

    tile_mod.TileContext._drain_and_barrier = _drain_and_barrier
    tile_mod.TileContext._drain_patched = True


def _build_module():
    import concourse.bass as bass
    import concourse.tile as tile
    from concourse import mybir

    _patch_drain()
    f8e3 = mybir.dt.float8e3
    bf16 = mybir.dt.bfloat16
    f32 = mybir.dt.float32
    EXP = mybir.ActivationFunctionType.Exp

    # Skip the const-AP init barrier: its memsets (gpsimd, done ~3.5us in)
    # only feed the ACT bias read >10us later, and removing the barrier
    # lets the input DMAs issue ~2us earlier.
    _orig_barrier = bass.Bass.all_engine_barrier
    bass.Bass.all_engine_barrier = lambda self: None
    try:
        nc = bass.Bass()
    finally:
        bass.Bass.all_engine_barrier = _orig_barrier
    xt_in = nc.dram_tensor("xt", [128, NBLK, 4, 128], f8e3, kind="ExternalInput")
    xn_in = nc.dram_tensor("xn", [128, BPC, NBB, 2, HALF], f8e3, kind="ExternalInput")
    r_in = nc.dram_tensor("r", [128, 4, H], bf16, kind="ExternalInput")
    pool_out = nc.dram_tensor("pool", [H, BPC * 2, HALF], f32, kind="ExternalOutput")

    with tile.TileContext(nc) as tc:
        with (
            tc.tile_pool(name="xtp", bufs=1) as xtp,
            tc.tile_pool(name="xnp", bufs=1) as xnp,
            tc.tile_pool(name="consts", bufs=1) as consts,
            tc.tile_pool(name="ep", bufs=1) as ep,
            tc.tile_pool(name="pss", bufs=1, space="PSUM") as pss,
            tc.tile_pool(name="psp", bufs=1, space="PSUM") as psp,
        ):
            r_sb = consts.tile([128, 4, H], bf16)
            nc.scalar.dma_start(out=r_sb, in_=r_in[:, :, :])

            xt_sb = xtp.tile([128, NBLK, 4, 128], f8e3, name="xt_sb")
            xn_sb = xnp.tile([128, BPC, NBB, 2, HALF], f8e3, name="xn_sb")
            # two HWDGE rings, chunks ordered to feed the half-batch
            # pipeline: S0a S0b P0a P0b S1a S1b P1a P1b
            for lo, hi in [(0, 8), (8, 16), (16, 24), (24, 32)]:
                nc.sync.dma_start(out=xt_sb[:, lo:hi], in_=xt_in[:, lo:hi])
            for b, lo, hi in [(0, 0, 8), (0, 8, 16), (1, 0, 8), (1, 8, 16)]:
                nc.scalar.dma_start(
                    out=xn_sb[:, b, lo:hi], in_=xn_in[:, b, lo:hi]
                )

            ps_s = [
                pss.tile([128, NBB * H], f32, name=f"ps_s{b}") for b in range(BPC)
            ]
            ps_p = [
                [psp.tile([H, HALF], f32, name=f"ps_p{b}_{h2}") for h2 in range(2)]
                for b in range(BPC)
            ]
            e_sb = [ep.tile([128, NBB, H], bf16, name=f"e{b}") for b in range(BPC)]
            out_sb = ep.tile([H, BPC * 2, HALF], f32, name="out_sb")

            def scores(b, hf):
                # E[t, h] = exp(x_t . R), accumulated per 128-token block
                for blk in range(hf * 8, hf * 8 + 8):
                    a = b * NBB + blk
                    dst = ps_s[b][:, blk * H : (blk + 1) * H]
                    for q in range(4):
                        nc.tensor.matmul(
                            dst,
                            xt_sb[:, a, q, :],
                            r_sb[:, q, :],
                            start=(q == 0),
                            stop=(q == 3),
                        )
                # one activation per half-batch: PSUM fp32 -> bf16 E
                nc.scalar.activation(
                    out=e_sb[b][:, hf * 8 : hf * 8 + 8, :],
                    in_=ps_s[b][:, hf * 64 : hf * 64 + 64].rearrange(
                        "p (i h) -> p i h", h=H
                    ),
                    func=EXP,
                )

            def pooling(b, hf):
                # pooled^T[h, c] = sum_t E[t, h] * xn[t, c]; the (b, h2)
                # accumulation group spans both hf calls (contiguous emission)
                for h2 in range(2):
                    for blk in range(hf * 8, hf * 8 + 8):
                        nc.tensor.matmul(
                            ps_p[b][h2],
                            e_sb[b][:, blk, :],
                            xn_sb[:, b, blk, h2, :],
                            start=(blk == 0),
                            stop=(blk == NBB - 1),
                        )

            def emit_out(b):
                for h2 in range(2):
                    nc.vector.tensor_copy(out_sb[:, b * 2 + h2, :], ps_p[b][h2])

            scores(0, 0)
            scores(0, 1)
            pooling(0, 0)
            pooling(0, 1)
            scores(1, 0)
            scores(1, 1)
            pooling(1, 0)
            pooling(1, 1)
            emit_out(0)
            emit_out(1)
            nc.scalar.dma_start(out=pool_out[:, :, :], in_=out_sb)

    return nc


def _get_module():
    if "nc" not in _cached:
        _cached["nc"] = _build_module()
    return _cached["nc"]


def _host_prep(cls, qkv_w):
    """R (scores projection) and the CLS token's own score, in fp64."""
    scale = HD ** -0.5
    c = cls.reshape(C).astype(np.float64)
    Wq = qkv_w[:C].astype(np.float64)
    Wk = qkv_w[C : 2 * C].astype(np.float64)
    q = (Wq @ c).reshape(H, HD)
    Wkh = Wk.reshape(H, HD, C)
    R = scale * np.einsum("hdc,hd->ch", Wkh, q)           # (C, H)
    k0 = Wk @ c
    s0 = scale * np.einsum("hd,hd->h", q, k0.reshape(H, HD))
    return R, s0


def _shard_images(x):
    """Build the two per-core fp8 images of x (see module docstring)."""
    import ml_dtypes

    f8 = ml_dtypes.float8_e3m4
    x8 = np.ascontiguousarray(x.reshape(B * N, C) * np.float32(XSC)).astype(f8)
    xts, xns = [], []
    for i in range(NCORES):
        xc = x8[i * TOK : (i + 1) * TOK]                   # (4096, 512)
        # xt[p, a, q, j] = 2x[a*128+j, q*128+p]
        xt = np.ascontiguousarray(
            xc.reshape(NBLK, 128, 4, 128).transpose(3, 0, 2, 1)
        )
        # xn[p, b, i, h2, j<256] = 2x[b*2048+i*128+p, h2*256+j]; j=256 -> 1
        xn = np.zeros((128, BPC, NBB, 2, HALF), dtype=f8)
        xn[..., :256] = xc.reshape(BPC, NBB, 128, 2, 256).transpose(2, 0, 1, 3, 4)
        xn[..., 256] = f8(1.0)
        xts.append(xt)
        xns.append(xn)
    return xts, xns


def _in_maps(inputs):
    import ml_dtypes

    x = np.asarray(inputs["x"], dtype=np.float32)
    cls = np.asarray(inputs["cls"], dtype=np.float32)
    qkv_w = np.asarray(inputs["qkv_w"], dtype=np.float32)
    R, _ = _host_prep(cls, qkv_w)
    r_img = np.ascontiguousarray(
        (R / XSC).reshape(4, 128, H).transpose(1, 0, 2)
    ).astype(ml_dtypes.bfloat16)
    xts, xns = _shard_images(x)
    return [{"xt": xts[i], "xn": xns[i], "r": r_img} for i in range(NCORES)]


def kernel(x, cls, qkv_w, proj_w, proj_b):
    from concourse.bass_utils import run_bass_kernel_spmd

    x = np.asarray(x, dtype=np.float32)
    cls = np.asarray(cls, dtype=np.float32)
    qkv_w = np.asarray(qkv_w, dtype=np.float32)
    proj_w = np.asarray(proj_w, dtype=np.float32)
    proj_b = np.asarray(proj_b, dtype=np.float32)

    nc = _get_module()
    res = run_bass_kernel_spmd(nc, _in_maps({"x": x, "cls": cls, "qkv_w": qkv_w}),
                               list(range(NCORES)))
    _cached["last_results"] = res

    R, s0 = _host_prep(cls, qkv_w)
    e0 = np.exp(s0)                                        # device exp is unbiased
    Wv = qkv_w[2 * C :].astype(np.float64)
    cf = cls.reshape(C).astype(np.float64)

    # pool[h, b*2+h2, j]: j<256 -> sum_t e_t * 2x[t, h2*256+j]; j=256 -> Z
    num = np.empty((B, H, C), dtype=np.float64)
    z = np.empty((B, H), dtype=np.float64)
    for i in range(NCORES):
        p = res.results[i]["pool"].astype(np.float64)      # (H, 4, HALF)
        for b in range(BPC):
            num[i * BPC + b, :, :256] = p[:, b * 2, :256] / XSC
            num[i * BPC + b, :, 256:] = p[:, b * 2 + 1, :256] / XSC
            z[i * BPC + b] = p[:, b * 2, 256]
    num += (e0[:, None] * cf[None, :])[None]
    z += e0[None]
    v = num / z[:, :, None]
    o = np.einsum("hdc,bhc->bhd", Wv.reshape(H, HD, C), v).reshape(B, C)
    y = o @ proj_w.T.astype(np.float64) + proj_b.astype(np.float64)
    return y.astype(np.float32)


# revision 23
# speedup vs baseline: 1.1714x; 1.1714x over previous
"""AttentionalJoin kernel for 8 Trainium2 NeuronCores.

Math: the reference builds full (M x M) self-attention over M = N+1 tokens
(CLS prepended) but returns only the CLS row of the projected output.  Only
the CLS query survives, so attention collapses to a softmax-weighted token
pooling:

    q       = Wq @ cls                       (per head h: q_h)
    score_t = scale * q_h . (Wk x_t)_h  =  x_t . R[:, h],   R = scale*Wk_h^T q_h
    p       = softmax over the M tokens
    pooled_h = sum_t p_t x_t                 (linearity: project AFTER pooling)
    out     = proj( concat_h Wv_h pooled_h ) + proj_b

Device design (per core, 2 batches of 2048 tokens):
  * x is uploaded TWICE in fp8-e3m4 (scaled by 2 to dodge the e3m4 subnormal
    cliff): once c-major (xt, the scores operand) and once t-major with an
    appended ones column (xn, the pooling operand + partition function Z).
    Dual-layout upload removes every on-chip transpose: the baseline spent
    ~60% of its time building X^T on the PE and copying PSUM->SBUF.
  * scores: per 128-token block, xt blocks are the PE's STATIONARY operand
    (fast-weight-load, 128-col fp8) against a tiny bf16 rhs R[128c, 8h],
    accumulating E-layout [128t, 8h] directly in PSUM.  Mixed-dtype matmul
    (fp8 weights x bf16 moving) is supported by the PE and keeps R at bf16
    precision for free.
  * exp: one ACT activation per batch over [128, 16*8] PSUM -> bf16 E.
  * pooling: E blocks [128t, 8h] are the stationary operand (bf16) against
    the streamed xn [128t, 260] halves (fp8), accumulating pooled^T [8, 260]
    over the 16 blocks of each batch.  Column 256 is the ones column -> Z.
  * tail (CLS contribution, normalize, Wv head-mix, proj, bias: ~10 MFLOP)
    runs on host in fp64.

Sharding: data-parallel over the batch dim, 2 batches per core.
"""

import numpy as np

H = 8
C = 512
HD = C // H
B = 16
N = 2048
NCORES = 8
BPC = B // NCORES          # batches per core
TOK = BPC * N              # tokens per core (4096)
NBLK = TOK // 128          # 128-token blocks per core (32)
NBB = N // 128             # blocks per batch (16)
HALF = 260                 # pooling rhs half width: 256 x + 1 ones + 3 pad
XSC = 2.0                  # x is stored as 2*x in e3m4
MAX_DRAIN_WAITS = 1        # this walrus rejects instructions w/ >1 sem wait

_cached = {}


def _patch_drain():
    """The container's walrus codegen rejects instructions carrying more
    than one sem wait ("Too many sync wait commands").  Split extra waits
    onto dedicated same-engine NOPs, which preserves semantics (engine
    queues are in-order)."""
    import concourse.tile as tile_mod
    from concourse import mybir
    from bass_rust import ScopedClock

    if getattr(tile_mod.TileContext, "_drain_patched", False):
        return

    orig_lower = tile_mod.TileContext._lower_ordered_insts

    def _lower_ordered_insts(self, ordered):
        nc = self.nc
        for bbname, insts in ordered.items():
            out = []
            for inst in insts:
                si = inst.sync_info
                if si is not None and si.on_wait and len(si.on_wait) > MAX_DRAIN_WAITS:
                    waits = list(si.on_wait)
                    extra, keep = waits[:-MAX_DRAIN_WAITS], waits[-MAX_DRAIN_WAITS:]
                    for w in extra:
                        nop = mybir.InstNoOp(
                            name=f"waitsplit-{nc.next_id()}",
                            engine=inst.engine,
                            ins=[],
                            outs=[],
                            bass_nofuse=True,
                            sync_info=mybir.SyncInfo(on_wait=[w], on_update=[]),
                            debug=inst.debug,
                        )
                        out.append(nop)
                    inst.sync_info = mybir.SyncInfo(
                        on_wait=keep, on_update=list(si.on_update)
                    )
                out.append(inst)
            ordered[bbname] = out
        return orig_lower(self, ordered)

    tile_mod.TileContext._lower_ordered_insts = _lower_ordered_insts

    def _drain_and_barrier(self, tick_clock, wait_clock):
        nc = self.nc
        probe = mybir.InstNoOp(
            name=f"drain-wait-probe-{nc.next_id()}",
            engine=mybir.EngineType.SP,
            ins=[],
            outs=[],
        )
        wait_clock.add_sem_waits(probe, ScopedClock({None: tick_clock.global_clock}))
        waits = list(probe.sync_info.on_wait) if probe.sync_info else []
        for i in range(0, len(waits), MAX_DRAIN_WAITS):
            chunk = waits[i : i + MAX_DRAIN_WAITS]
            nop = nc.sync.nop(nofuse=True, hint="drain_wait")
            nop.ins.sync_info = mybir.SyncInfo(on_wait=chunk, on_update=[])
        nc.sync.drain()

        nc.all_engine_barrier()
        popped = nc._tile_sem_poison_stack.pop()
        assert popped is self._sem_poison
        nc.clear_and_free_semaphores(list(self.sems.allocated().values()))
        nc.all_engine_barrier()

    tile_mod.TileContext._drain_and_barrier = _drain_and_barrier
    tile_mod.TileContext._drain_patched = True


def _build_module():
    import concourse.bass as bass
    import concourse.tile as tile
    from concourse import mybir

    _patch_drain()
    f8e3 = mybir.dt.float8e3
    bf16 = mybir.dt.bfloat16
    f32 = mybir.dt.float32
    EXP = mybir.ActivationFunctionType.Exp

    # Skip the const-AP init barrier: its memsets (gpsimd, done ~3.5us in)
    # only feed the ACT bias read >10us later, and removing the barrier
    # lets the input DMAs issue ~2us earlier.
    _orig_barrier = bass.Bass.all_engine_barrier
    bass.Bass.all_engine_barrier = lambda self: None
    try:
        nc = bass.Bass()
    finally:
        bass.Bass.all_engine_barrier = _orig_barrier
    xt_in = nc.dram_tensor("xt", [128, NBLK, 4, 128], f8e3, kind="ExternalInput")
    xn_in = nc.dram_tensor("xn", [128, BPC, NBB, 2, HALF], f8e3, kind="ExternalInput")
    r_in = nc.dram_tensor("r", [128, 4, H], bf16, kind="ExternalInput")
    pool_out = nc.dram_tensor("pool", [H, BPC * 2, HALF], f32, kind="ExternalOutput")

    with tile.TileContext(nc) as tc:
        with (
            tc.tile_pool(name="xtp", bufs=1) as xtp,
            tc.tile_pool(name="xnp", bufs=1) as xnp,
            tc.tile_pool(name="consts", bufs=1) as consts,
            tc.tile_pool(name="ep", bufs=1) as ep,
            tc.tile_pool(name="pss", bufs=1, space="PSUM") as pss,
            tc.tile_pool(name="psp", bufs=1, space="PSUM") as psp,
        ):
            r_sb = consts.tile([128, 4, H], bf16)
            nc.scalar.dma_start(out=r_sb, in_=r_in[:, :, :])

            xt_sb = xtp.tile([128, NBLK, 4, 128], f8e3, name="xt_sb")
            xn_sb = xnp.tile([128, BPC, NBB, 2, HALF], f8e3, name="xn_sb")
            # ONE HWDGE ring (SP) for all of x: a second ring steals packets
            # from the same SDMA engines and runs far slower when its host
            # engine is busy.  Chunks interleave xt/xn in exact compute
            # order: S0a S0b P0a P0b S1a S1b P1a P1b.
            nc.sync.dma_start(out=xt_sb[:, 0:8], in_=xt_in[:, 0:8])
            nc.sync.dma_start(out=xn_sb[:, 0, 0:8], in_=xn_in[:, 0, 0:8])
            nc.sync.dma_start(out=xt_sb[:, 8:16], in_=xt_in[:, 8:16])
            nc.sync.dma_start(out=xn_sb[:, 0, 8:16], in_=xn_in[:, 0, 8:16])
            nc.sync.dma_start(out=xt_sb[:, 16:24], in_=xt_in[:, 16:24])
            nc.sync.dma_start(out=xn_sb[:, 1, 0:8], in_=xn_in[:, 1, 0:8])
            nc.sync.dma_start(out=xt_sb[:, 24:32], in_=xt_in[:, 24:32])
            nc.sync.dma_start(out=xn_sb[:, 1, 8:16], in_=xn_in[:, 1, 8:16])

            ps_s = [
                pss.tile([128, NBB * H], f32, name=f"ps_s{b}") for b in range(BPC)
            ]
            ps_p = [
                [psp.tile([H, HALF], f32, name=f"ps_p{b}_{h2}") for h2 in range(2)]
                for b in range(BPC)
            ]
            e_sb = [ep.tile([128, NBB, H], bf16, name=f"e{b}") for b in range(BPC)]
            out_sb = ep.tile([H, BPC * 2, HALF], f32, name="out_sb")

            def scores(b, hf):
                # E[t, h] = exp(x_t . R), accumulated per 128-token block
                for blk in range(hf * 8, hf * 8 + 8):
                    a = b * NBB + blk
                    dst = ps_s[b][:, blk * H : (blk + 1) * H]
                    for q in range(4):
                        nc.tensor.matmul(
                            dst,
                            xt_sb[:, a, q, :],
                            r_sb[:, q, :],
                            start=(q == 0),
                            stop=(q == 3),
                        )
                # one activation per half-batch: PSUM fp32 -> bf16 E
                nc.scalar.activation(
                    out=e_sb[b][:, hf * 8 : hf * 8 + 8, :],
                    in_=ps_s[b][:, hf * 64 : hf * 64 + 64].rearrange(
                        "p (i h) -> p i h", h=H
                    ),
                    func=EXP,
                )

            def pooling(b, hf):
                # pooled^T[h, c] = sum_t E[t, h] * xn[t, c]; the (b, h2)
                # accumulation group spans both hf calls (contiguous emission)
                for h2 in range(2):
                    for blk in range(hf * 8, hf * 8 + 8):
                        nc.tensor.matmul(
                            ps_p[b][h2],
                            e_sb[b][:, blk, :],
                            xn_sb[:, b, blk, h2, :],
                            start=(blk == 0),
                            stop=(blk == NBB - 1),
                            skip_group_check=True,
                        )

            def emit_out(b):
                for h2 in range(2):
                    nc.vector.tensor_copy(out_sb[:, b * 2 + h2, :], ps_p[b][h2])

            scores(0, 0)
            pooling(0, 0)
            scores(0, 1)
            pooling(0, 1)
            emit_out(0)
            scores(1, 0)
            pooling(1, 0)
            scores(1, 1)
            pooling(1, 1)
            emit_out(1)
            nc.scalar.dma_start(out=pool_out[:, :, :], in_=out_sb)

    return nc


def _get_module():
    if "nc" not in _cached:
        _cached["nc"] = _build_module()
    return _cached["nc"]


def _host_prep(cls, qkv_w):
    """R (scores projection) and the CLS token's own score, in fp64."""
    scale = HD ** -0.5
    c = cls.reshape(C).astype(np.float64)
    Wq = qkv_w[:C].astype(np.float64)
    Wk = qkv_w[C : 2 * C].astype(np.float64)
    q = (Wq @ c).reshape(H, HD)
    Wkh = Wk.reshape(H, HD, C)
    R = scale * np.einsum("hdc,hd->ch", Wkh, q)           # (C, H)
    k0 = Wk @ c
    s0 = scale * np.einsum("hd,hd->h", q, k0.reshape(H, HD))
    return R, s0


def _shard_images(x):
    """Build the two per-core fp8 images of x (see module docstring)."""
    import ml_dtypes

    f8 = ml_dtypes.float8_e3m4
    x8 = np.ascontiguousarray(x.reshape(B * N, C) * np.float32(XSC)).astype(f8)
    xts, xns = [], []
    for i in range(NCORES):
        xc = x8[i * TOK : (i + 1) * TOK]                   # (4096, 512)
        # xt[p, a, q, j] = 2x[a*128+j, q*128+p]
        xt = np.ascontiguousarray(
            xc.reshape(NBLK, 128, 4, 128).transpose(3, 0, 2, 1)
        )
        # xn[p, b, i, h2, j<256] = 2x[b*2048+i*128+p, h2*256+j]; j=256 -> 1
        xn = np.zeros((128, BPC, NBB, 2, HALF), dtype=f8)
        xn[..., :256] = xc.reshape(BPC, NBB, 128, 2, 256).transpose(2, 0, 1, 3, 4)
        xn[..., 256] = f8(1.0)
        xts.append(xt)
        xns.append(xn)
    return xts, xns


def _in_maps(inputs):
    import ml_dtypes

    x = np.asarray(inputs["x"], dtype=np.float32)
    cls = np.asarray(inputs["cls"], dtype=np.float32)
    qkv_w = np.asarray(inputs["qkv_w"], dtype=np.float32)
    R, _ = _host_prep(cls, qkv_w)
    r_img = np.ascontiguousarray(
        (R / XSC).reshape(4, 128, H).transpose(1, 0, 2)
    ).astype(ml_dtypes.bfloat16)
    xts, xns = _shard_images(x)
    return [{"xt": xts[i], "xn": xns[i], "r": r_img} for i in range(NCORES)]


def kernel(x, cls, qkv_w, proj_w, proj_b):
    from concourse.bass_utils import run_bass_kernel_spmd

    x = np.asarray(x, dtype=np.float32)
    cls = np.asarray(cls, dtype=np.float32)
    qkv_w = np.asarray(qkv_w, dtype=np.float32)
    proj_w = np.asarray(proj_w, dtype=np.float32)
    proj_b = np.asarray(proj_b, dtype=np.float32)

    nc = _get_module()
    res = run_bass_kernel_spmd(nc, _in_maps({"x": x, "cls": cls, "qkv_w": qkv_w}),
                               list(range(NCORES)))
    _cached["last_results"] = res

    R, s0 = _host_prep(cls, qkv_w)
    e0 = np.exp(s0)                                        # device exp is unbiased
    Wv = qkv_w[2 * C :].astype(np.float64)
    cf = cls.reshape(C).astype(np.float64)

    # pool[h, b*2+h2, j]: j<256 -> sum_t e_t * 2x[t, h2*256+j]; j=256 -> Z
    num = np.empty((B, H, C), dtype=np.float64)
    z = np.empty((B, H), dtype=np.float64)
    for i in range(NCORES):
        p = res.results[i]["pool"].astype(np.float64)      # (H, 4, HALF)
        for b in range(BPC):
            num[i * BPC + b, :, :256] = p[:, b * 2, :256] / XSC
            num[i * BPC + b, :, 256:] = p[:, b * 2 + 1, :256] / XSC
            z[i * BPC + b] = p[:, b * 2, 256]
    num += (e0[:, None] * cf[None, :])[None]
    z += e0[None]
    v = num / z[:, :, None]
    o = np.einsum("hdc,bhc->bhd", Wv.reshape(H, HD, C), v).reshape(B, C)
    y = o @ proj_w.T.astype(np.float64) + proj_b.astype(np.float64)
    return y.astype(np.float32)
